# revision 34
# baseline (speedup 1.0000x reference)
"""Trainium2 Bass kernel for a dense fp32 MultiHeadAttention layer.

Problem (hardcoded): B=4, T=S=2048, C=1024, 16 heads x 64 dims, fp32.
  q = query @ Wq.T + bq ; k,v likewise
  scores = (q k^T) * D**-0.5 + attn_mask + padding_mask
  out = softmax(scores) @ v -> reshape -> @ Wout.T + bout

Sharding over 8 NeuronCores: core c = (batch b = c//2, head-group g = c%2).
Each core handles one batch and 8 of the 16 heads:
  - column-parallel q/k/v projections (512-dim slice of the projections)
  - attention for its 8 heads (full T x S, on-chip scores)
  - row-parallel out_proj producing a partial (T, C) output
Host sums the two partials per batch and adds the bias terms
(bout + bv @ Wout.T, which commutes with softmax since sum(weights)=1).

Numerics: activations/weights are staged in bf16 (halves DMA + SBUF;
matmuls accumulate in fp32 PSUM).  Verified end-to-end rel-err ~6e-3
against an f64 reference, well inside the 2e-2 gate.

Schedule (per core).  The exp() ACTIVATEs on the scalar engine are the
steady-state pacer (~1.1us per 128x1024 tile, 64 per t-chunk), so the
main loop keeps that stream dense and feeds every other matmul through
its gaps:
  phase 1: K projection (all S), then V projection, then Q projection
    for t-chunk 0 (xq staged through a double-buffered SBUF stash).
  main loop over t-chunks: per head-pair pr: 16x [2 score matmuls ->
    exp -> 1 filler matmul], then PV with the denominator ones-column,
    then normalize (batched reciprocal + one one-hot broadcast matmul
    for both heads).  "Filler" matmuls are the NEXT t-chunk's q
    projection and the PREVIOUS t-chunk's out-projection, so the PE
    never drains at t-chunk boundaries and exp never starves.

Layout notes (per core):
  - host ships transposed activations xT (C, T) so projections need no
    on-device transposes:
      qT/kT (f-major): psum = wT_chunk.T @ xT_chunk   (f on partitions)
      v (s-major):     psum = xT_chunk.T @ wT_chunk   (s on partitions)
  - scores are computed transposed, (s on partitions, t free):
      psc = kT_chunk.T @ qT   so softmax's s-reduction becomes a matmul
  - v is stored with a ones column per head (65 wide); the PV matmul
      outT = [v|1].T @ exp(scores^T)
    then yields numerator rows 0..63 and the softmax denominator in row 64.
  - normalization: batched recip of both heads' row 64, broadcast across
    partitions with a two-row one-hot matmul (e2.T @ rr), one DVE
    copy+mul per head.
"""

import os
import numpy as np
import ml_dtypes

import concourse.bass as bass
import concourse.mybir as mybir
import concourse.tile as tile
from concourse import bacc
from concourse.bass_utils import run_bass_kernel_spmd

# ---- problem constants ----
B, T, S, C = 4, 2048, 2048, 1024
H, D = 16, 64
NCORES = 8
F = 512            # per-core projection slice (8 heads x 64)
SCALE = D ** -0.5
P = 128
TCH = 512          # t-chunk (score free dim)
NTC = T // TCH     # 4
NSC = S // P       # 16 s-chunks
NFC = F // P       # 4 f-chunks per core
NCC = C // P       # 8 contraction chunks
HW = 65            # v width per head incl. ones column

FP32 = mybir.dt.float32
DT = mybir.dt.bfloat16   # staging dtype for all matmul operands
NPDT = ml_dtypes.bfloat16

LAST_EXEC_NS = None
LAST_TRACE = None
LAST_NC = None
LAST_IN_MAPS = None


def build(use_mask: bool):
    nc = bacc.Bacc("TRN2", target_bir_lowering=False, debug=False,
                   num_devices=NCORES)

    xq = nc.dram_tensor("xq", [C, T], DT, kind="ExternalInput")
    xk = nc.dram_tensor("xk", [C, S], DT, kind="ExternalInput")
    xv = nc.dram_tensor("xv", [C, S], DT, kind="ExternalInput")
    wq = nc.dram_tensor("wq", [C, F], DT, kind="ExternalInput")
    wk = nc.dram_tensor("wk", [C, F], DT, kind="ExternalInput")
    wv = nc.dram_tensor("wv", [C, F], DT, kind="ExternalInput")
    wo = nc.dram_tensor("wo", [F, C], DT, kind="ExternalInput")
    bqr = nc.dram_tensor("bqr", [P, NFC], FP32, kind="ExternalInput")
    bkr = nc.dram_tensor("bkr", [P, NFC], FP32, kind="ExternalInput")
    if use_mask:
        emask = nc.dram_tensor("emask", [S, T], DT, kind="ExternalInput")
    out = nc.dram_tensor("out", [T, C], FP32, kind="ExternalOutput")

    xq_r = xq.rearrange("(cc p) t -> p cc t", p=P)
    xk_r = xk.rearrange("(cc p) s -> p cc s", p=P)
    xv_r = xv.rearrange("(cc p) s -> p cc s", p=P)
    wq_r = wq.rearrange("(cc p) f -> p cc f", p=P)
    wk_r = wk.rearrange("(cc p) f -> p cc f", p=P)
    wv_r = wv.rearrange("(cc p) f -> p cc f", p=P)
    wo_r = wo.rearrange("(dc p) f -> p dc f", p=P)

    with tile.TileContext(nc) as tc:
        with (
            tc.tile_pool(name="const", bufs=1) as cp,
            tc.tile_pool(name="qt", bufs=2) as qpool,
            tc.tile_pool(name="at", bufs=2) as apool,
            tc.tile_pool(name="xqs", bufs=2) as xpool,
            tc.tile_pool(name="mains", bufs=4) as ms,
            tc.tile_pool(name="xbig", bufs=3) as xb,
            tc.tile_pool(name="maino", bufs=2) as mo,
        ):
            wq_sb = cp.tile([P, NCC, F], DT, tag="wq")
            wo_sb = cp.tile([P, NFC, C], DT, tag="wo")
            bq_sb = cp.tile([P, NFC], FP32, tag="bq")
            bk_sb = cp.tile([P, NFC], FP32, tag="bk")
            e2_sb = cp.tile([P, D], DT, tag="e2")
            rr32 = cp.tile([P, 2 * TCH], FP32, tag="rr32")
            rr32b = cp.tile([P, 2 * TCH], FP32, tag="rr32b")
            rrB = cp.tile([P, 2 * TCH], DT, tag="rrB")
            pbcB = cp.tile([P, TCH], DT, tag="pbcB")
            kT_sb = cp.tile([P, NFC, S], DT, tag="kT")
            v_sb = cp.tile([P, NSC, 8 * HW], DT, tag="v")
            # ping-pong exp slabs: phase k writes slab k%2 while the PV of
            # phase k-1 drains the other one, so exp never waits on PV
            expT = [cp.tile([P, NSC, 2 * TCH], DT, tag=f"expT{i}",
                            name=f"expT{i}")
                    for i in range(2)]


            # one-hot broadcast stationary: only partition-0 row is 1; both
            # heads' reciprocal rows live side by side on partition 0 of
            # rrB (all engine ops start at partition 0).  bf16 tiles are
            # initialized by broadcast-copy from fp32 scratch columns
            # (DVE casts on write).
            one_sb = cp.tile([P, 1], FP32, tag="one")
            zero_sb = cp.tile([P, 1], FP32, tag="zero")
            nc.any.memset(one_sb[:], 1.0)
            nc.any.memset(zero_sb[:], 0.0)
            nc.vector.tensor_copy(e2_sb[:],
                                  zero_sb[:, 0:1].to_broadcast(e2_sb.shape))
            nc.vector.tensor_copy(e2_sb[0:1, 0:D],
                                  one_sb[0:1, 0:1].to_broadcast((1, D)))
            nc.vector.tensor_copy(rrB[:],
                                  zero_sb[:, 0:1].to_broadcast(rrB.shape))
            ones_dst = v_sb[:].rearrange("p s (h e) -> p s h e", e=HW)[:, :, :, D]
            nc.vector.tensor_copy(ones_dst,
                                  one_sb[:, 0:1].to_broadcast(ones_dst.shape))

            stash = [None] * NTC
            qT = [None] * NTC

            # ---------------- phase 1: k/v/q0 projections --------------
            with (
                tc.tile_pool(name="ph1w", bufs=1) as wp,
                tc.tile_pool(name="ph1p", bufs=8, space="PSUM") as pp,
            ):
                wk_sb = wp.tile([P, NCC, F], DT, tag="wk")
                wv_sb = wp.tile([P, NCC, F], DT, tag="wv")
                # whole-tile DMAs: dma_start issue costs ~600ns serialized
                # on the sync sequencer, so batch aggressively (wk and the
                # first xk are split only to unblock the first matmul)
                nc.sync.dma_start(wk_sb[:, 0:4, :], wk_r[:, 0:4, :])
                # K projection over full S
                for sw in range(S // TCH):
                    psk = [pp.tile([P, TCH], FP32, tag="pp", name="psk")
                           for _ in range(NFC)]
                    xk_t = xb.tile([P, NCC, TCH], DT, tag="xch", name="xk_t")
                    if sw == 0:
                        nc.sync.dma_start(xk_t[:, 0:1, :], xk_r[:, 0:1, 0:TCH])
                        nc.sync.dma_start(xk_t[:, 1:8, :], xk_r[:, 1:8, 0:TCH])
                        nc.sync.dma_start(wk_sb[:, 4:8, :], wk_r[:, 4:8, :])
                        nc.sync.dma_start(bk_sb[:], bkr[:])
                        nc.sync.dma_start(bq_sb[:], bqr[:])
                    else:
                        nc.sync.dma_start(xk_t[:],
                                          xk_r[:, :, sw * TCH:(sw + 1) * TCH])
                    for cc in range(NCC):
                        for fc in range(NFC):
                            nc.tensor.matmul(
                                psk[fc][:],
                                wk_sb[:, cc, fc * P:(fc + 1) * P],
                                xk_t[:, cc, :],
                                start=(cc == 0), stop=(cc == NCC - 1))
                    for fc in range(NFC):
                        nc.vector.tensor_scalar_add(
                            kT_sb[:, fc, sw * TCH:(sw + 1) * TCH],
                            psk[fc][:], bk_sb[:, fc:fc + 1])
                # queue the rest of the weight/stash traffic behind K's
                nc.sync.dma_start(wv_sb[:], wv_r[:])
                nc.sync.dma_start(wq_sb[:], wq_r[:])
                for t in (0, 1):
                    stash[t] = xpool.tile([P, NCC, TCH], DT, tag="stash",
                                          name="stash")
                    nc.sync.dma_start(stash[t][:],
                                      xq_r[:, :, t * TCH:(t + 1) * TCH])
                # V projection over full S (s-major, with ones column)
                for sw in range(S // TCH):
                    psv = [pp.tile([P, TCH], FP32, tag="pp", name="psv")
                           for _ in range(4)]
                    xv_t = xb.tile([P, NCC, TCH], DT, tag="xch", name="xv_t")
                    nc.sync.dma_start(xv_t[:],
                                      xv_r[:, :, sw * TCH:(sw + 1) * TCH])
                    for cc in range(NCC):
                        for ss in range(4):
                            nc.tensor.matmul(
                                psv[ss][:],
                                xv_t[:, cc, ss * P:(ss + 1) * P],
                                wv_sb[:, cc, :],
                                start=(cc == 0), stop=(cc == NCC - 1))
                    for ss in range(4):
                        sc = sw * 4 + ss
                        dst = v_sb[:, sc, :].rearrange(
                            "p (h e) -> p h e", e=HW)[:, :, 0:D]
                        src = psv[ss][:].rearrange("p (h e) -> p h e", e=D)
                        nc.vector.tensor_copy(dst, src)
                nc.sync.dma_start(wo_sb[:], wo_r[:])
                # Q projection for t-chunk 0
                qT[0] = qpool.tile([P, NFC, TCH], DT, tag="qT", name="qT")
                for fc in range(NFC):
                    psq = pp.tile([P, TCH], FP32, tag="pp", name="psq")
                    for cc in range(NCC):
                        nc.tensor.matmul(
                            psq[:],
                            wq_sb[:, cc, fc * P:(fc + 1) * P],
                            stash[0][:, cc, :],
                            start=(cc == 0), stop=(cc == NCC - 1))
                    nc.vector.tensor_scalar_add(
                        qT[0][:, fc, :], psq[:], bq_sb[:, fc:fc + 1])

            # ---------------- phase 2: main loop over t-chunks ----------
            with (
                tc.tile_pool(name="pscore", bufs=2, space="PSUM") as pscp,
                tc.tile_pool(name="ppv", bufs=2, space="PSUM") as ppvp,
                tc.tile_pool(name="pgen", bufs=2, space="PSUM") as pgp,
            ):
                attnT = [None] * NTC

                def emit_outp(tcx):
                    tp0 = tcx * TCH
                    for tw in range(4):
                        for fh in range(2):
                            po = pgp.tile([P, TCH], FP32, tag="pgen",
                                          name="po")
                            for dc in range(NFC):
                                nc.tensor.matmul(
                                    po[:],
                                    attnT[tcx][:, dc, tw * P:(tw + 1) * P],
                                    wo_sb[:, dc, fh * TCH:(fh + 1) * TCH],
                                    start=(dc == 0), stop=(dc == NFC - 1))
                            ob = mo.tile([P, TCH], FP32, tag="ob", name="ob")
                            nc.vector.tensor_copy(ob[:], po[:])
                            nc.sync.dma_start(
                                out[tp0 + tw * P:tp0 + (tw + 1) * P,
                                    fh * TCH:(fh + 1) * TCH],
                                ob[:])

                def emit_pv(k, ppvs, sc):
                    tcx, pr = divmod(k, NFC)
                    eT = expT[k % 2]
                    for h in range(2):
                        hh = pr * 2 + h
                        nc.tensor.matmul(
                            ppvs[h][:],
                            v_sb[:, sc, hh * HW:(hh + 1) * HW],
                            eT[:, sc, h * TCH:(h + 1) * TCH],
                            start=(sc == 0), stop=(sc == NSC - 1))

                def emit_norm(k, ppvs, pbc_pool=None, pbc_tag="pgen"):
                    tcx, pr = divmod(k, NFC)
                    # normalization: both heads' denominator rows staged
                    # side by side on partition 0, one batched reciprocal,
                    # then per-head broadcast matmuls
                    for h in range(2):
                        nc.vector.tensor_copy(
                            rr32[0:1, h * TCH:(h + 1) * TCH],
                            ppvs[h][D:D + 1, :])
                    nc.vector.reciprocal_approx_fast(rr32b[0:1, :],
                                                     rr32[0:1, :])
                    nc.vector.tensor_copy(rrB[0:1, :], rr32b[0:1, :])
                    pool = pbc_pool if pbc_pool is not None else pgp
                    pbc = pool.tile([P, TCH], FP32, tag=pbc_tag, name="pbc")
                    for h in range(2):
                        nc.tensor.matmul(pbc[h * D:(h + 1) * D, :],
                                         e2_sb[:],
                                         rrB[:, h * TCH:(h + 1) * TCH],
                                         start=True, stop=True)
                    # stage the broadcast through SBUF bf16 so the
                    # normalize mul is a same-dtype 2-byte DVE op
                    nc.vector.tensor_copy(pbcB[:], pbc[:])
                    for h in range(2):
                        dst = attnT[tcx][h * D:(h + 1) * D, pr, :]
                        nc.vector.tensor_copy(dst, ppvs[h][0:D, :])
                        nc.vector.tensor_mul(dst, dst,
                                             pbcB[h * D:(h + 1) * D, :])

                # software pipeline over the 16 (tcx, pr) phases: phase k
                # runs its 16 score pairs + exps while the PV of phase k-1
                # is interleaved between them, so the ACT exp stream never
                # drains at phase boundaries
                fillers, fi = [], [0]

                def pump(n):
                    j = 0
                    while j < n and fi[0] < len(fillers):
                        fillers[fi[0]]()
                        fi[0] += 1
                        j += 1

                ppvs_prev = None
                for k in range(NTC * NFC):
                    tcx, pr = divmod(k, NFC)
                    if pr == 0:
                        # new t-chunk: flush leftovers, rebuild filler list
                        pump(len(fillers))
                        fillers, fi = [], [0]
                        if tcx + 2 < NTC:
                            stash[tcx + 2] = xpool.tile(
                                [P, NCC, TCH], DT, tag="stash", name="stash")
                            nc.sync.dma_start(
                                stash[tcx + 2][:],
                                xq_r[:, :, (tcx + 2) * TCH:(tcx + 3) * TCH])
                        attnT[tcx] = apool.tile([P, NFC, TCH], DT,
                                                tag="attnT", name="attnT")
                        if tcx + 1 < NTC:
                            qT[tcx + 1] = qpool.tile([P, NFC, TCH], DT,
                                                     tag="qT", name="qT")
                            qTn, st = qT[tcx + 1], stash[tcx + 1]

                            def q_mm(cc, fc, box, qTn=qTn, st=st):
                                if cc == 0:
                                    box["ps"] = pgp.tile(
                                        [P, TCH], FP32, tag="pgen",
                                        name="psq")
                                nc.tensor.matmul(
                                    box["ps"][:],
                                    wq_sb[:, cc, fc * P:(fc + 1) * P],
                                    st[:, cc, :],
                                    start=(cc == 0), stop=(cc == NCC - 1))
                                if cc == NCC - 1:
                                    nc.vector.tensor_scalar_add(
                                        qTn[:, fc, :], box["ps"][:],
                                        bq_sb[:, fc:fc + 1])

                            for fc in range(NFC):
                                box = {}
                                for cc in range(NCC):
                                    fillers.append(
                                        lambda cc=cc, fc=fc, box=box:
                                        q_mm(cc, fc, box))
                        if tcx >= 1:
                            if tcx + 1 >= NTC:
                                # no qproj fillers ahead of outp: delay outp
                                # by one phase so the last norm of tcx-1
                                # (emitted at the end of this tcx's first
                                # phase) lands before outp reads attnT
                                fillers.extend([lambda: None] * NSC)
                            atp, tp0 = attnT[tcx - 1], (tcx - 1) * TCH

                            def o_mm(dc, tw, fh, box, atp=atp, tp0=tp0):
                                if dc == 0:
                                    box["po"] = pgp.tile(
                                        [P, TCH], FP32, tag="pgen", name="po")
                                nc.tensor.matmul(
                                    box["po"][:],
                                    atp[:, dc, tw * P:(tw + 1) * P],
                                    wo_sb[:, dc, fh * TCH:(fh + 1) * TCH],
                                    start=(dc == 0), stop=(dc == NFC - 1))
                                if dc == NFC - 1:
                                    ob = mo.tile([P, TCH], FP32, tag="ob",
                                                 name="ob")
                                    nc.vector.tensor_copy(ob[:], box["po"][:])
                                    nc.sync.dma_start(
                                        out[tp0 + tw * P:tp0 + (tw + 1) * P,
                                            fh * TCH:(fh + 1) * TCH],
                                        ob[:])

                            for tw in range(4):
                                for fh in range(2):
                                    box = {}
                                    for dc in range(NFC):
                                        fillers.append(
                                            lambda dc=dc, tw=tw, fh=fh,
                                            box=box: o_mm(dc, tw, fh, box))

                    qTc = qT[tcx]
                    eT = expT[k % 2]
                    # tiles for the PREVIOUS phase's PV, which is emitted
                    # interleaved into this phase's score stream
                    ppvs_prev = ([ppvp.tile([HW, TCH], FP32, tag="ppv",
                                            name="ppv") for _ in range(2)]
                                 if k >= 1 else None)
                    t0 = tcx * TCH
                    for sc in range(NSC):
                        psc = pscp.tile([P, 2, TCH], FP32, tag="pscore",
                                        name="psc")
                        for h in range(2):
                            nc.tensor.matmul(
                                psc[:, h, :],
                                kT_sb[h * D:(h + 1) * D, pr,
                                      sc * P:(sc + 1) * P],
                                qTc[h * D:(h + 1) * D, pr, :],
                                start=True, stop=True)
                        if ppvs_prev is not None:
                            emit_pv(k - 1, ppvs_prev, sc)
                        nc.scalar.activation(
                            eT[:, sc, :],
                            psc[:].rearrange("p a b -> p (a b)"),
                            mybir.ActivationFunctionType.Exp, scale=SCALE)
                        if use_mask:
                            em_t = ms.tile([P, TCH], DT, tag="emk")
                            nc.sync.dma_start(
                                em_t[:],
                                emask[sc * P:(sc + 1) * P, t0:t0 + TCH])
                            for h in range(2):
                                nc.vector.tensor_mul(
                                    eT[:, sc, h * TCH:(h + 1) * TCH],
                                    eT[:, sc, h * TCH:(h + 1) * TCH],
                                    em_t[:])
                        pump(1)
                    if ppvs_prev is not None:
                        emit_norm(k - 1, ppvs_prev)

                # drain: PV + norm of the last phase, leftovers, final outp.
                # The first two outp chains accumulate their dc=0..2 steps
                # (which only need already-normalized pr slices) inside the
                # PV loop so the PE stays busy through the final normalize.
                klast = NTC * NFC - 1
                tp0 = (NTC - 1) * TCH
                ppvs_last = [ppvp.tile([HW, TCH], FP32, tag="ppv",
                                       name="ppv") for _ in range(2)]
                early = [(tw, fh, pgp.tile([P, TCH], FP32, tag="pgen",
                                           name="po"))
                         for tw, fh in ((0, 0), (0, 1))]
                esteps = [(tw, fh, po, dc) for tw, fh, po in early
                          for dc in range(3)]
                for sc in range(NSC):
                    emit_pv(klast, ppvs_last, sc)
                    pump(1)
                    if sc % 2 == 1 and esteps:
                        tw, fh, po, dc = esteps.pop(0)
                        nc.tensor.matmul(
                            po[:],
                            attnT[NTC - 1][:, dc, tw * P:(tw + 1) * P],
                            wo_sb[:, dc, fh * TCH:(fh + 1) * TCH],
                            start=(dc == 0), stop=False)
                emit_norm(klast, ppvs_last, pbc_pool=pscp, pbc_tag="pscore")
                pump(len(fillers))
                for tw, fh, po in early:
                    nc.tensor.matmul(
                        po[:],
                        attnT[NTC - 1][:, 3, tw * P:(tw + 1) * P],
                        wo_sb[:, 3, fh * TCH:(fh + 1) * TCH],
                        start=False, stop=True)
                    ob = mo.tile([P, TCH], FP32, tag="ob", name="ob")
                    nc.vector.tensor_copy(ob[:], po[:])
                    nc.sync.dma_start(
                        out[tp0 + tw * P:tp0 + (tw + 1) * P,
                            fh * TCH:(fh + 1) * TCH],
                        ob[:])
                for tw in range(4):
                    for fh in range(2):
                        if (tw, fh) in ((0, 0), (0, 1)):
                            continue
                        po = pgp.tile([P, TCH], FP32, tag="pgen", name="po")
                        for dc in range(NFC):
                            nc.tensor.matmul(
                                po[:],
                                attnT[NTC - 1][:, dc, tw * P:(tw + 1) * P],
                                wo_sb[:, dc, fh * TCH:(fh + 1) * TCH],
                                start=(dc == 0), stop=(dc == NFC - 1))
                        ob = mo.tile([P, TCH], FP32, tag="ob", name="ob")
                        nc.vector.tensor_copy(ob[:], po[:])
                        nc.sync.dma_start(
                            out[tp0 + tw * P:tp0 + (tw + 1) * P,
                                fh * TCH:(fh + 1) * TCH],
                            ob[:])

    nc.compile()
    return nc


_CACHE = {}


def _get(use_mask: bool):
    if use_mask not in _CACHE:
        _CACHE[use_mask] = build(use_mask)
    return _CACHE[use_mask]


def kernel(query, key, value, attn_mask, key_padding_mask,
           Wq, bq, Wk, bk, Wv, bv, Wout, bout):
    global LAST_EXEC_NS, LAST_TRACE
    query = np.asarray(query, np.float32)
    key = np.asarray(key, np.float32)
    value = np.asarray(value, np.float32)
    attn_mask = np.asarray(attn_mask, np.float32)
    key_padding_mask = np.asarray(key_padding_mask)
    Wq, bq = np.asarray(Wq, np.float32), np.asarray(bq, np.float32)
    Wk, bk = np.asarray(Wk, np.float32), np.asarray(bk, np.float32)
    Wv, bv = np.asarray(Wv, np.float32), np.asarray(bv, np.float32)
    Wout, bout = np.asarray(Wout, np.float32), np.asarray(bout, np.float32)

    use_mask = bool(np.any(attn_mask)) or bool(np.any(key_padding_mask))
    nc = _get(use_mask)

    def bcast(x):
        return np.ascontiguousarray(x).astype(NPDT)

    in_maps = []
    for c in range(NCORES):
        b, g = divmod(c, 2)
        gs = g * F
        im = {
            "xq": bcast(query[b].T),
            "xk": bcast(key[b].T),
            "xv": bcast(value[b].T),
            "wq": bcast(Wq[gs:gs + F, :].T),
            "wk": bcast(Wk[gs:gs + F, :].T),
            "wv": bcast(Wv[gs:gs + F, :].T),
            "wo": bcast(Wout[:, gs:gs + F].T),
            "bqr": np.ascontiguousarray(bq[gs:gs + F].reshape(NFC, P).T),
            "bkr": np.ascontiguousarray(bk[gs:gs + F].reshape(NFC, P).T),
        }
        if use_mask:
            m = attn_mask.T.astype(np.float64).copy()
            m[key_padding_mask[b], :] = -np.inf
            im["emask"] = np.exp(m).astype(np.float32).astype(NPDT)
        in_maps.append(im)

    global LAST_NC, LAST_IN_MAPS
    LAST_NC, LAST_IN_MAPS = nc, in_maps
    res = run_bass_kernel_spmd(nc, in_maps, list(range(NCORES)))
    LAST_EXEC_NS = res.exec_time_ns
    LAST_TRACE = res.instructions_and_trace[1] if res.instructions_and_trace else None
    globals()["LAST_INSTS"] = (res.instructions_and_trace[0]
                               if res.instructions_and_trace else None)

    extra = (bv @ Wout.T + bout).astype(np.float32)
    outp = np.empty((B, T, C), np.float32)
    for b in range(B):
        outp[b] = res.results[2 * b]["out"] + res.results[2 * b + 1]["out"] + extra
    return outp
